# revision 5
# baseline (speedup 1.0000x reference)
import os
import sys
from contextlib import ExitStack

import numpy as np

for _p in ("/opt/trn_rl_repo",):
    if os.path.isdir(_p) and _p not in sys.path:
        sys.path.insert(0, _p)

# Problem (nn_PosDecoder): out[n,l] = sum_c src[n,l,:128] . (table[1+c]*sqrt(128))
#   = src[n,l,:128] . colsum  where colsum = sqrt(128) * sum(table[1:], axis=0).
# Shard table rows across 8 cores; each core computes a partial colsum and a
# partial (N,L) output row; host sums the 8 partial rows.
N, L, M = 16, 100, 256
F = 128
N_LOC = 100001
N_CORES = 8
R = (N_LOC - 1) // N_CORES  # 12500 table rows per core
TOK = N * L  # 1600
NBLK = R // 128  # 97 full 128-row blocks
TAIL = R - NBLK * 128  # 84
SCALE = float(np.sqrt(F))

CW = 8 * F  # chunk width: 8 blocks = 1024 cols
N_SC = 6    # sync-queue chunks -> acc_D (DVE adds)
N_AC = 5    # act-queue chunks  -> acc_P (POOL adds) + last one to acc_D
N_SING = NBLK - (N_SC + N_AC) * 8  # 9 single blocks -> PE directly

_BUILT = None


def _build():
    import concourse.bass as bass
    import concourse.tile as tile
    from concourse import bacc, mybir

    nc = bacc.Bacc("TRN2", target_bir_lowering=False, debug=False,
                   num_devices=N_CORES)
    f32 = mybir.dt.float32
    table = nc.dram_tensor("table_slice", (R, F), f32, kind="ExternalInput").ap()
    srcT = nc.dram_tensor("srcT", (F, TOK), f32, kind="ExternalInput").ap()
    out = nc.dram_tensor("out", (1, TOK), f32, kind="ExternalOutput").ap()

    with tile.TileContext(nc) as tc, ExitStack() as ctx:
        sb = ctx.enter_context(tc.tile_pool(name="sb", bufs=1))
        schunks = ctx.enter_context(tc.tile_pool(name="schunks", bufs=3))
        achunks = ctx.enter_context(tc.tile_pool(name="achunks", bufs=3))
        ssing = ctx.enter_context(tc.tile_pool(name="ssing", bufs=2))
        asing = ctx.enter_context(tc.tile_pool(name="asing", bufs=3))
        parts = ctx.enter_context(tc.tile_pool(name="parts", bufs=2))
        psum1 = ctx.enter_context(
            tc.tile_pool(name="psum1", bufs=1, space=bass.MemorySpace.PSUM))
        psumv = ctx.enter_context(
            tc.tile_pool(name="psumv", bufs=4, space=bass.MemorySpace.PSUM))

        ones = sb.tile([128, 1], f32)
        nc.gpsimd.memset(ones[:], SCALE)  # folds the sqrt(F) scale into colsum
        srcT_sb = sb.tile([128, TOK], f32)
        out_sb = sb.tile([1, TOK], f32)
        acc_D = sb.tile([128, CW], f32)
        acc_P = sb.tile([128, CW], f32)
        tailt = sb.tile([TAIL, F], f32)

        main = table[0:NBLK * 128, :].rearrange("(p t) f -> p (t f)", p=128)

        # Block-range plan: interleave S/A chunks for arrival order.
        # S0 A0 S1 A1 S2 A2 S3 A3 S4 A4 S5 then 9 singles (blocks 88..96).
        s_cols = [(2 * i) * CW for i in range(N_SC)]       # S0..S5
        a_cols = [(2 * i + 1) * CW for i in range(N_AC)]   # A0..A4
        sing0 = (N_SC + N_AC) * CW                         # col of block 88

        # --- sync HWDGE queue: S0 (direct into acc_D), S1..S5, srcT left,
        #     2 singles
        nc.sync.dma_start(acc_D[:], main[:, s_cols[0]:s_cols[0] + CW])
        s_tiles = []
        for c in s_cols[1:]:
            ch = schunks.tile([128, CW], f32)
            nc.sync.dma_start(ch[:], main[:, c:c + CW])
            s_tiles.append(ch)
        nc.sync.dma_start(srcT_sb[:, :800], srcT[:, :800])
        sing_s = []
        for i in range(2):
            t = ssing.tile([128, F], f32)
            nc.sync.dma_start(t[:], main[:, sing0 + i * F:sing0 + (i + 1) * F])
            sing_s.append(t)

        # --- act HWDGE queue: tailt, A0 (direct into acc_P), A1..A4,
        #     srcT right, 7 singles
        nc.scalar.dma_start(tailt[:], table[NBLK * 128:R, :])
        nc.scalar.dma_start(acc_P[:], main[:, a_cols[0]:a_cols[0] + CW])
        a_tiles = []
        for c in a_cols[1:]:
            ch = achunks.tile([128, CW], f32)
            nc.scalar.dma_start(ch[:], main[:, c:c + CW])
            a_tiles.append(ch)
        nc.scalar.dma_start(srcT_sb[:, 800:], srcT[:, 800:])
        sing_a = []
        for i in range(2, N_SING):
            t = asing.tile([128, F], f32)
            nc.scalar.dma_start(t[:], main[:, sing0 + i * F:sing0 + (i + 1) * F])
            sing_a.append(t)

        # --- POOL: acc_P += A1..A3
        for ch in a_tiles[:3]:
            nc.gpsimd.tensor_add(acc_P[:], acc_P[:], ch[:])

        # --- DVE: acc_D += S1..S3, fold_P, acc_D += S4, A4, S5, fold_D
        part_P = parts.tile([128, F], f32)
        part_D = parts.tile([128, F], f32)
        for ch in s_tiles[:3]:
            nc.vector.tensor_add(acc_D[:], acc_D[:], ch[:])
        nc.vector.tensor_reduce(
            part_P[:], acc_P.rearrange("p (b f) -> p f b", f=F),
            axis=mybir.AxisListType.X, op=mybir.AluOpType.add)
        nc.vector.tensor_add(acc_D[:], acc_D[:], s_tiles[3][:])
        nc.vector.tensor_add(acc_D[:], acc_D[:], a_tiles[3][:])
        nc.vector.tensor_add(acc_D[:], acc_D[:], s_tiles[4][:])
        nc.vector.tensor_reduce(
            part_D[:], acc_D.rearrange("p (b f) -> p f b", f=F),
            axis=mybir.AxisListType.X, op=mybir.AluOpType.add)

        # --- PE: one PSUM accumulation group -> colsum (128,1)
        cps = psum1.tile([128, 1], f32)
        nc.tensor.matmul(cps[:], tailt[:], ones[:TAIL, :], start=True,
                         stop=False)
        nc.tensor.matmul(cps[:], part_P[:], ones[:], start=False, stop=False)
        for t in sing_a:
            nc.tensor.matmul(cps[:], t[:], ones[:], start=False, stop=False)
        for t in sing_s:
            nc.tensor.matmul(cps[:], t[:], ones[:], start=False, stop=False)
        nc.tensor.matmul(cps[:], part_D[:], ones[:], start=False, stop=True)
        colsum = sb.tile([128, 1], f32)
        nc.vector.tensor_copy(colsum[:], cps[:])

        # --- out_row = colsum^T @ srcT -> (1, 1600)
        for j in range(0, TOK, 512):
            w = min(512, TOK - j)
            pv = psumv.tile([1, 512], f32)
            nc.tensor.matmul(pv[:1, :w], colsum[:], srcT_sb[:, j:j + w],
                             start=True, stop=True)
            nc.vector.tensor_copy(out_sb[:, j:j + w], pv[:1, :w])
        nc.sync.dma_start(out[:], out_sb[:])

    nc.compile()
    return nc


def make_in_maps(src, lookup_table):
    src_f = np.asarray(src, dtype=np.float32).reshape(TOK, M)[:, :F]
    srcT_np = np.ascontiguousarray(src_f.T)  # (128, 1600)
    tab = np.asarray(lookup_table, dtype=np.float32)
    in_maps = []
    for k in range(N_CORES):
        sl = np.ascontiguousarray(tab[1 + k * R:1 + (k + 1) * R, :])
        in_maps.append({"table_slice": sl, "srcT": srcT_np})
    return in_maps


def kernel(src=None, ds=None, lookup_table=None, **_):
    global _BUILT
    if _BUILT is None:
        _BUILT = _build()
    from concourse import bass_utils

    in_maps = make_in_maps(src, lookup_table)
    res = bass_utils.run_bass_kernel_spmd(_BUILT, in_maps,
                                          core_ids=list(range(N_CORES)))
    parts = [next(iter(r.values())).reshape(-1) for r in res.results]
    total = np.sum(np.stack(parts, 0), axis=0, dtype=np.float64)
    return total.astype(np.float32).reshape(N, L)


# revision 6
# speedup vs baseline: 1.1138x; 1.1138x over previous
import os
import sys
from contextlib import ExitStack

import numpy as np

for _p in ("/opt/trn_rl_repo",):
    if os.path.isdir(_p) and _p not in sys.path:
        sys.path.insert(0, _p)

# Problem (nn_PosDecoder): out[n,l] = sum_c src[n,l,:128] . (table[1+c]*sqrt(128))
#   = src[n,l,:128] . colsum  where colsum = sqrt(128) * sum(table[1:], axis=0).
# Shard table rows across 8 cores; each core computes a partial colsum and a
# partial (N,L) output row; host sums the 8 partial rows.
N, L, M = 16, 100, 256
F = 128
N_LOC = 100001
N_CORES = 8
R = (N_LOC - 1) // N_CORES  # 12500 table rows per core
TOK = N * L  # 1600
NBLK = R // 128  # 97 full 128-row blocks
TAIL = R - NBLK * 128  # 84
SCALE = float(np.sqrt(F))

CW = 8 * F   # wide chunk: 8 blocks = 1024 cols
N_W = 12     # 12 wide chunks (96 blocks) -> acc via DVE adds; block 96 -> PE

_BUILT = None


def _build():
    import concourse.bass as bass
    import concourse.tile as tile
    from concourse import bacc, mybir

    nc = bacc.Bacc("TRN2", target_bir_lowering=False, debug=False,
                   num_devices=N_CORES)
    f32 = mybir.dt.float32
    table = nc.dram_tensor("table_slice", (R, F), f32, kind="ExternalInput").ap()
    srcT = nc.dram_tensor("srcT", (F, TOK), f32, kind="ExternalInput").ap()
    out = nc.dram_tensor("out", (1, TOK), f32, kind="ExternalOutput").ap()

    with tile.TileContext(nc) as tc, ExitStack() as ctx:
        sb = ctx.enter_context(tc.tile_pool(name="sb", bufs=1))
        schunks = ctx.enter_context(tc.tile_pool(name="schunks", bufs=3))
        achunks = ctx.enter_context(tc.tile_pool(name="achunks", bufs=3))
        parts = ctx.enter_context(tc.tile_pool(name="parts", bufs=1))
        psum1 = ctx.enter_context(
            tc.tile_pool(name="psum1", bufs=1, space=bass.MemorySpace.PSUM))
        psumv = ctx.enter_context(
            tc.tile_pool(name="psumv", bufs=4, space=bass.MemorySpace.PSUM))

        ones = sb.tile([128, 1], f32)
        nc.gpsimd.memset(ones[:], SCALE)  # folds the sqrt(F) scale into colsum
        srcT_sb = sb.tile([128, TOK], f32)
        out_sb = sb.tile([1, TOK], f32)
        acc = sb.tile([128, CW], f32)
        g0 = sb.tile([128, F], f32)
        tailt = sb.tile([TAIL, F], f32)

        main = table[0:NBLK * 128, :].rearrange("(p t) f -> p (t f)", p=128)
        sing0 = N_W * CW  # col offset of block 96

        # --- sync HWDGE queue: W0 (direct into acc), W2,W4,..W10, then
        #     srcT slices s0 (cols 0:512) and s2 (cols 1024:1536), then out.
        nc.sync.dma_start(acc[:], main[:, 0:CW])
        s_tiles = []
        for i in range(1, 6):
            c = (2 * i) * CW
            ch = schunks.tile([128, CW], f32)
            nc.sync.dma_start(ch[:], main[:, c:c + CW])
            s_tiles.append(ch)
        nc.sync.dma_start(srcT_sb[:, 0:512], srcT[:, 0:512])
        nc.sync.dma_start(srcT_sb[:, 1024:1536], srcT[:, 1024:1536])

        # --- act HWDGE queue: W1,W3,..W11, single block g0, tailt, then
        #     srcT slices s1 (512:1024) and s3 (1536:1600).
        a_tiles = []
        for i in range(6):
            c = (2 * i + 1) * CW
            ch = achunks.tile([128, CW], f32)
            nc.scalar.dma_start(ch[:], main[:, c:c + CW])
            a_tiles.append(ch)
        nc.scalar.dma_start(g0[:], main[:, sing0:sing0 + F])
        nc.scalar.dma_start(tailt[:], table[NBLK * 128:R, :])
        nc.scalar.dma_start(srcT_sb[:, 512:1024], srcT[:, 512:1024])
        nc.scalar.dma_start(srcT_sb[:, 1536:1600], srcT[:, 1536:1600])

        # --- DVE: acc += the 11 chunks (interleaved arrival order), then fold
        order = []
        for i in range(6):
            if i > 0:
                order.append(s_tiles[i - 1])
            order.append(a_tiles[i])
        for ch in order:
            nc.vector.tensor_add(acc[:], acc[:], ch[:])
        part = parts.tile([128, F], f32)
        nc.vector.tensor_reduce(
            part[:], acc.rearrange("p (b f) -> p f b", f=F),
            axis=mybir.AxisListType.X, op=mybir.AluOpType.add)

        # --- PE: one PSUM accumulation group -> colsum (128,1)
        cps = psum1.tile([128, 1], f32)
        nc.tensor.matmul(cps[:], g0[:], ones[:], start=True, stop=False)
        nc.tensor.matmul(cps[:], tailt[:], ones[:TAIL, :], start=False,
                         stop=False)
        nc.tensor.matmul(cps[:], part[:], ones[:], start=False, stop=True)
        colsum = sb.tile([128, 1], f32)
        nc.vector.tensor_copy(colsum[:], cps[:])

        # --- out_row = colsum^T @ srcT -> (1, 1600)
        for j in range(0, TOK, 512):
            w = min(512, TOK - j)
            pv = psumv.tile([1, 512], f32)
            nc.tensor.matmul(pv[:1, :w], colsum[:], srcT_sb[:, j:j + w],
                             start=True, stop=True)
            nc.vector.tensor_copy(out_sb[:, j:j + w], pv[:1, :w])
        nc.sync.dma_start(out[:], out_sb[:])

    nc.compile()
    return nc


def make_in_maps(src, lookup_table):
    src_f = np.asarray(src, dtype=np.float32).reshape(TOK, M)[:, :F]
    srcT_np = np.ascontiguousarray(src_f.T)  # (128, 1600)
    tab = np.asarray(lookup_table, dtype=np.float32)
    in_maps = []
    for k in range(N_CORES):
        sl = np.ascontiguousarray(tab[1 + k * R:1 + (k + 1) * R, :])
        in_maps.append({"table_slice": sl, "srcT": srcT_np})
    return in_maps


def kernel(src=None, ds=None, lookup_table=None, **_):
    global _BUILT
    if _BUILT is None:
        _BUILT = _build()
    from concourse import bass_utils

    in_maps = make_in_maps(src, lookup_table)
    res = bass_utils.run_bass_kernel_spmd(_BUILT, in_maps,
                                          core_ids=list(range(N_CORES)))
    parts = [next(iter(r.values())).reshape(-1) for r in res.results]
    total = np.sum(np.stack(parts, 0), axis=0, dtype=np.float64)
    return total.astype(np.float32).reshape(N, L)
